# revision 24
# baseline (speedup 1.0000x reference)
"""Multi-head attention (b=1, n=2048, d_model=1024, 16 heads x 64) on 8 TRN2
NeuronCores, head-parallel tensor parallelism: each core computes 2 heads end
to end (qkv projection for its heads, attention, and its slice of the output
projection); the 8 partial outputs (rank-128 slices of the out-proj
contraction) are summed on the host along with b_out and the v-bias
correction (bv @ W_out, constant through softmax-normalized attention).

Device kernel per core (bf16 compute, f32 accumulation in PSUM):
  A) load x [2048,1024] f32, cast to bf16 on GPSIMD, XBAR DMA-transpose
     (SBUF->SBUF) -> xT; the PE never touches the transpose
  B) qT/kT = W^T x^T (q bias folded into the PSUM->SBUF copy via a
     per-partition tensor_scalar add; k bias dropped -- it cancels in
     softmax; v bias moved to host), v natural per tile with a ones
     column per head for the softmax row-sums
  C) per 512-query chunk: S^T = K Q^T per head (64-row contraction, no
     zero padding) -> exp on ACT -> P^T bf16; A^T_aug = V_aug^T P^T
     accumulated over key tiles in PSUM; softmax denominators ride in
     row 64 of the AV psum
  D) normalize with reciprocal_approx_fast + rank-1 PE broadcast (both
     emitted off the PE critical path), partial_out = A^T^T W_out -> f16
"""

import os
import sys

sys.path.insert(0, "/opt/trn_rl_repo")

import numpy as np
import ml_dtypes

import concourse.bass as bass
import concourse.tile as tile
from concourse import bacc, mybir
from concourse.bass_utils import run_bass_kernel_spmd
from concourse.masks import make_identity

F32 = mybir.dt.float32
F16 = mybir.dt.float16
BF16 = mybir.dt.bfloat16

N = 2048          # sequence length
D = 1024          # d_model
H_PER_CORE = 2    # heads per core
DH = 64           # head dim
C = H_PER_CORE * DH   # per-core qkv width = 128
N_CORES = 8
P = 128
N_TILES = N // P      # 16
D_TILES = D // P      # 8
I_CHUNK = 512         # query-chunk width for attention
N_ICHUNKS = N // I_CHUNK  # 4

_CACHE = {}


def build_graph():
    nc = bacc.Bacc()

    x_ext = nc.declare_dram_parameter("x", [N, D], F32, isOutput=False)
    wq_ext = nc.declare_dram_parameter("wq", [D, C], BF16, isOutput=False)
    wk_ext = nc.declare_dram_parameter("wk", [D, C], BF16, isOutput=False)
    wv_ext = nc.declare_dram_parameter("wv", [D, C], BF16, isOutput=False)
    wo_ext = nc.declare_dram_parameter("wo", [C, D], BF16, isOutput=False)
    bq_ext = nc.declare_dram_parameter("bq", [1, C], BF16, isOutput=False)
    out_ext = nc.declare_dram_parameter("out", [N, D], F16, isOutput=True)
    debug = bool(os.environ.get("KDEBUG"))
    if debug:
        qt_dbg = nc.declare_dram_parameter("qt_dbg", [P, N], BF16, isOutput=True)
        kt_dbg = nc.declare_dram_parameter("kt_dbg", [P, N], BF16, isOutput=True)
        v_dbg = nc.declare_dram_parameter(
            "v_dbg", [P, N_TILES * 2 * (DH + 1)], BF16, isOutput=True)
        pt_dbg = nc.declare_dram_parameter(
            "pt_dbg", [P, 2 * I_CHUNK], BF16, isOutput=True)
        at_dbg = nc.declare_dram_parameter("at_dbg", [P, N], BF16, isOutput=True)
        den_dbg = nc.declare_dram_parameter(
            "den_dbg", [1, 8 * I_CHUNK], F32, isOutput=True)
        rin_dbg = nc.declare_dram_parameter(
            "rin_dbg", [1, 8 * I_CHUNK], F32, isOutput=True)

    EXP = mybir.ActivationFunctionType.Exp
    MULT = mybir.AluOpType.mult

    with tile.TileContext(nc) as tc:
        with (
            tc.tile_pool(name="persist", bufs=1) as persist,
            tc.tile_pool(name="xload", bufs=10) as xload,
            tc.tile_pool(name="xcast", bufs=6) as xcast,
            tc.tile_pool(name="pt", bufs=33) as ptpool,
            tc.tile_pool(name="small", bufs=2) as small,
            tc.tile_pool(name="rbcp", bufs=2) as rbcp,
            tc.tile_pool(name="outsb", bufs=3) as outsb,
            tc.tile_pool(name="ps_s", bufs=2, space="PSUM") as ps_s,
            tc.tile_pool(name="ps_av", bufs=2, space="PSUM") as ps_av,
            tc.tile_pool(name="ps_mm", bufs=2, space="PSUM") as ps_mm,
        ):
            ones_col = persist.tile([1, P], BF16)
            nc.gpsimd.memset(ones_col, 1.0)
            ones_row = persist.tile([1, I_CHUNK], BF16)
            nc.gpsimd.memset(ones_row, 1.0)

            # --- weights / biases to SBUF ---
            wq_sb = persist.tile([P, D_TILES, C], BF16)
            wk_sb = persist.tile([P, D_TILES, C], BF16)
            wv_sb = persist.tile([P, D_TILES, C], BF16)
            wo_sb = persist.tile([C, D], BF16)
            bq_sb = persist.tile([1, C], BF16)

            # warm the ACT exp table before the first real activation
            warm = persist.tile([1, 1], BF16)
            nc.scalar.activation(warm[:], ones_col[0:1, 0:1], EXP)

            xT = persist.tile([P, D_TILES, N], BF16)
            qT = persist.tile([P, N], BF16)   # rows: h0 dims 0:64, h1 64:128
            kT = persist.tile([P, N], BF16)   # same packing, no zero pad
            # per key-tile: [v_h0 (64) | 1 | v_h1 (64) | 1]
            v_sb = persist.tile([P, N_TILES, 2 * (DH + 1)], BF16)
            nc.gpsimd.memset(v_sb[:, :, DH:DH + 1], 1.0)
            nc.gpsimd.memset(v_sb[:, :, 2 * DH + 1:2 * DH + 2], 1.0)
            aT = persist.tile([P, N], BF16)   # normalized A^T, heads stacked

            pts = {c: [] for c in range(N_ICHUNKS)}
            osb_ref = {}
            if debug:
                dendbg_sb = persist.tile([1, 8 * I_CHUNK], F32)
                rindbg_sb = persist.tile([1, 8 * I_CHUNK], F32)

            def emit_score(c, j):
                """S^T tile for both heads of (chunk c, key tile j) + exp."""
                sps = ps_s.tile([P, 2 * I_CHUNK], F32, tag="s", name=f"s{c}_{j}")
                ccols = slice(c * I_CHUNK, (c + 1) * I_CHUNK)
                jcols = slice(j * P, (j + 1) * P)
                for h in range(H_PER_CORE):
                    hr = slice(h * DH, (h + 1) * DH)
                    nc.tensor.matmul(
                        sps[:, h * I_CHUNK:(h + 1) * I_CHUNK],
                        kT[hr, jcols], qT[hr, ccols], start=True, stop=True)
                pt = ptpool.tile([P, 2 * I_CHUNK], BF16, tag="pt",
                                 name=f"pt{c}_{j}")
                nc.scalar.activation(pt[:], sps[:], EXP)
                pts[c].append(pt)

            def emit_av_step(avps, c, j):
                for h in range(H_PER_CORE):
                    nc.tensor.matmul(
                        avps[h][:],
                        v_sb[:, j, h * (DH + 1):(h + 1) * (DH + 1)],
                        pts[c][j][:, h * I_CHUNK:(h + 1) * I_CHUNK],
                        start=(j == 0), stop=(j == N_TILES - 1))

            def emit_norm_recip(c, avps):
                """DVE: pull both heads' denominator rows (psum partition 64)
                to a partition-0 SBUF row, then one fast reciprocal + cast.
                reciprocal_approx_fast silently ignores a psum partition
                offset, so the copy is required for correctness."""
                den = small.tile([1, 2 * I_CHUNK], F32, tag="den",
                                 name=f"den{c}")
                for h in range(H_PER_CORE):
                    nc.vector.tensor_copy(
                        out=den[:, h * I_CHUNK:(h + 1) * I_CHUNK],
                        in_=avps[h][DH:DH + 1, :])
                rinv = small.tile([1, 2 * I_CHUNK], F32, tag="rinv",
                                  name=f"rinv{c}")
                nc.vector.reciprocal_approx_fast(rinv[:], den[:])
                rbf = small.tile([1, 2 * I_CHUNK], BF16, tag="rbf",
                                 name=f"rbf{c}")
                nc.vector.tensor_copy(out=rbf[:], in_=rinv[:])
                if debug:
                    i0 = 2 * c * I_CHUNK
                    nc.vector.tensor_copy(
                        out=dendbg_sb[:, i0:i0 + 2 * I_CHUNK], in_=den[:])
                    nc.vector.tensor_copy(
                        out=rindbg_sb[:, i0:i0 + 2 * I_CHUNK], in_=rinv[:])
                return rbf

            def emit_norm_mult(c, h, avps, rbf):
                """PE rank-1 broadcast of 1/denom, then DVE row scale -> aT."""
                rbc_ps = ps_mm.tile([P, I_CHUNK], F32, tag="mm",
                                    name=f"rbc{c}_{h}")
                nc.tensor.matmul(
                    rbc_ps[:], ones_col[:],
                    rbf[:, h * I_CHUNK:(h + 1) * I_CHUNK],
                    start=True, stop=True)
                rbc_sb = rbcp.tile([P, I_CHUNK], F32, tag="rbc",
                                   name=f"rbcs{c}_{h}")
                nc.vector.tensor_copy(out=rbc_sb[:], in_=rbc_ps[:])
                ccols = slice(c * I_CHUNK, (c + 1) * I_CHUNK)
                nc.vector.tensor_tensor(
                    aT[h * DH:(h + 1) * DH, ccols], avps[h][0:DH, :],
                    rbc_sb[0:DH, :], MULT)

            def emit_outproj_step(c, k):
                """One of 8 out-proj matmuls for chunk c (4 q-tiles x 2),
                with its own f16 staging half and immediate DMA out."""
                iblk = c * 4 + k // 2
                nn = k % 2
                ops = ps_mm.tile([P, I_CHUNK], F32, tag="mm",
                                 name=f"op{c}_{k}")
                nc.tensor.matmul(
                    ops[:], aT[:, iblk * P:(iblk + 1) * P],
                    wo_sb[:, nn * I_CHUNK:(nn + 1) * I_CHUNK],
                    start=True, stop=True)
                osb = outsb.tile([P, I_CHUNK], F16, tag="osb",
                                 name=f"osb{c}_{k}")
                nc.vector.tensor_copy(out=osb[:], in_=ops[:])
                dma_eng = (nc.sync, nc.gpsimd)[(iblk * 2 + nn) % 2]
                dma_eng.dma_start(
                    out_ext[iblk * P:(iblk + 1) * P,
                            nn * I_CHUNK:(nn + 1) * I_CHUNK], osb[:])

            # --- first two x tiles in flight, then weights on the sync
            # queue (first-use order, wo last), then the remaining loads.
            # The scalar queue carries only the XBAR transposes so they
            # never sit behind loads in a queue.
            xf_tiles = {}

            def issue_xload(t):
                xf = xload.tile([P, D], F32, tag="xf", name=f"xf{t}")
                eng = (nc.sync, nc.gpsimd)[t % 2]
                eng.dma_start(xf[:], x_ext[t * P:(t + 1) * P, :])
                xf_tiles[t] = xf

            issue_xload(0)
            issue_xload(1)
            issue_xload(2)
            issue_xload(3)
            nc.sync.dma_start(
                wv_sb[:], wv_ext[:].rearrange("(o p) c -> p o c", p=P))
            nc.sync.dma_start(
                wk_sb[:], wk_ext[:].rearrange("(o p) c -> p o c", p=P))
            nc.sync.dma_start(
                wq_sb[:], wq_ext[:].rearrange("(o p) c -> p o c", p=P))
            nc.sync.dma_start(bq_sb[:], bq_ext[:])
            nc.sync.dma_start(wo_sb[:], wo_ext[:])

            # --- phase A+B as two decoupled stages emitted one group ahead:
            # x-pipeline (load -> cast on ACT/Pool -> XBAR transpose) feeds
            # compute (v/q/k projections + chunk-0 scores + smeared chunk-0
            # AV). Casts stay off the DVE so its drain copies never block a
            # producer; ACT casts for group g+1 are emitted before group g's
            # exps so the scalar stream never holds up the x pipeline.
            def emit_xloads(g):
                for t in range(4 * g, 4 * g + 4):
                    if t not in xf_tiles:
                        issue_xload(t)

            def emit_xpipe(g):
                for t in range(4 * g, 4 * g + 4):
                    xb = xcast.tile([P, D], BF16, tag="xb", name=f"xb{t}")
                    nc.vector.tensor_copy(out=xb[:], in_=xf_tiles[t][:])
                    nc.scalar.dma_start_transpose(
                        xT[:, :, t * P:(t + 1) * P], xb[:])

            def emit_compute(g):
                for t in range(4 * g, 4 * g + 4):
                    psv = ps_mm.tile([P, C], F32, tag="mm", name=f"psv{t}")
                    for do in range(D_TILES):
                        nc.tensor.matmul(
                            psv[:], xT[:, do, t * P:(t + 1) * P],
                            wv_sb[:, do, :],
                            start=(do == 0), stop=(do == D_TILES - 1))
                    nc.vector.tensor_copy(
                        out=v_sb[:, t, :].rearrange(
                            "p (h x) -> p h x", h=2)[:, :, 0:DH],
                        in_=psv[:].rearrange("p (h x) -> p h x", h=2))
                cols = slice(g * I_CHUNK, (g + 1) * I_CHUNK)
                psq = ps_mm.tile([P, I_CHUNK], F32, tag="mm", name=f"psq{g}")
                for do in range(D_TILES):
                    nc.tensor.matmul(
                        psq[:], wq_sb[:, do, :], xT[:, do, cols],
                        start=(do == 0), stop=False)
                nc.tensor.matmul(
                    psq[:], bq_sb[:], ones_row[:], start=False, stop=True)
                nc.vector.tensor_copy(out=qT[:, cols], in_=psq[:])
                psk = ps_mm.tile([P, I_CHUNK], F32, tag="mm", name=f"psk{g}")
                for do in range(D_TILES):
                    nc.tensor.matmul(
                        psk[:], wk_sb[:, do, :], xT[:, do, cols],
                        start=(do == 0), stop=(do == D_TILES - 1))
                nc.vector.tensor_copy(out=kT[:, cols], in_=psk[:])
                for j in range(4 * g, 4 * g + 4):
                    emit_score(0, j)
                if g >= 1:
                    for j in range(4 * (g - 1), 4 * (g - 1) + 4):
                        emit_av_step(avps0, 0, j)

            avps0 = [ps_av.tile([DH + 1, I_CHUNK], F32, tag="av",
                                name=f"av0_{h}") for h in range(H_PER_CORE)]
            for g in range(4):
                emit_xloads(g)
            emit_xpipe(0)
            emit_xpipe(1)
            emit_compute(0)
            emit_xpipe(2)
            emit_compute(1)
            emit_xpipe(3)
            emit_compute(2)
            emit_compute(3)

            # --- window 1: scores(1) with the tail of AV(0), norm(0), then
            # AV(1) j0..7 with out-proj(0) as filler.
            emit_score(1, 0)
            emit_score(1, 1)
            for i, j in enumerate((12, 13, 14, 15)):
                emit_av_step(avps0, 0, j)
                emit_score(1, 2 + i)
            rbf0 = emit_norm_recip(0, avps0)
            emit_score(1, 6)
            emit_score(1, 7)
            for h in range(H_PER_CORE):
                emit_norm_mult(0, h, avps0, rbf0)
            avps1 = [ps_av.tile([DH + 1, I_CHUNK], F32, tag="av",
                                name=f"av1_{h}") for h in range(H_PER_CORE)]
            for j in range(8):
                emit_score(1, 8 + j)
                emit_av_step(avps1, 1, j)
                emit_outproj_step(0, j)

            # --- window 2: scores(2) with the rest of AV(1)
            for j in range(0, 3):
                emit_score(2, j)
            for j in range(3, 11):
                emit_score(2, j)
                emit_av_step(avps1, 1, j + 5)
            for j in range(11, N_TILES):
                emit_score(2, j)
            rbf1 = emit_norm_recip(1, avps1)

            # --- window 3: scores(3) with AV(2), norm(1) + outproj(1)
            for j in range(0, 2):
                emit_score(3, j)
            for h in range(H_PER_CORE):
                emit_norm_mult(1, h, avps1, rbf1)
            avps2 = [ps_av.tile([DH + 1, I_CHUNK], F32, tag="av",
                                name=f"av2_{h}") for h in range(H_PER_CORE)]
            k_op = 0
            for j in range(2, N_TILES):
                emit_score(3, j)
                emit_av_step(avps2, 2, j - 2)
                if k_op < 8:
                    emit_outproj_step(1, k_op)
                    k_op += 1
            for j in range(N_TILES - 2, N_TILES):
                emit_av_step(avps2, 2, j)
            rbf2 = emit_norm_recip(2, avps2)

            # --- final: norm+outproj(2), AV(3), norm(3), outproj(3)
            for h in range(H_PER_CORE):
                emit_norm_mult(2, h, avps2, rbf2)
            avps3 = [ps_av.tile([DH + 1, I_CHUNK], F32, tag="av",
                                name=f"av3_{h}") for h in range(H_PER_CORE)]
            k_op = 0
            for j in range(N_TILES):
                emit_av_step(avps3, 3, j)
                if j >= 2 and k_op < 8:
                    emit_outproj_step(2, k_op)
                    k_op += 1
            rbf3 = emit_norm_recip(3, avps3)
            for h in range(H_PER_CORE):
                emit_norm_mult(3, h, avps3, rbf3)
            for k in range(8):
                emit_outproj_step(3, k)
            if debug:
                nc.sync.dma_start(qt_dbg[:], qT[:])
                nc.sync.dma_start(kt_dbg[:], kT[:])
                nc.sync.dma_start(
                    v_dbg[:], v_sb[:].rearrange("p t c -> p (t c)"))
                nc.sync.dma_start(pt_dbg[:], pts[0][0][:])
                nc.sync.dma_start(at_dbg[:], aT[:])
                nc.sync.dma_start(den_dbg[:], dendbg_sb[:])
                nc.sync.dma_start(rin_dbg[:], rindbg_sb[:])
    nc.compile()
    return nc


def _shard_inputs(x, W_qkv, b_qkv, W_out):
    x2d = np.ascontiguousarray(x.reshape(N, D), dtype=np.float32)
    Wr = np.asarray(W_qkv, dtype=np.float32).reshape(D, 3, 16, DH)
    br = np.asarray(b_qkv, dtype=np.float32).reshape(3, 16, DH)
    Wo = np.asarray(W_out, dtype=np.float32)
    scale = 1.0 / np.sqrt(DH)
    bf = ml_dtypes.bfloat16
    in_maps = []
    for c in range(N_CORES):
        hs = slice(2 * c, 2 * c + 2)
        in_maps.append({
            "x": x2d,
            "wq": np.ascontiguousarray(
                (Wr[:, 0, hs, :].reshape(D, C) * scale).astype(bf)),
            "wk": np.ascontiguousarray(Wr[:, 1, hs, :].reshape(D, C).astype(bf)),
            "wv": np.ascontiguousarray(Wr[:, 2, hs, :].reshape(D, C).astype(bf)),
            "wo": np.ascontiguousarray(Wo[c * C:(c + 1) * C, :].astype(bf)),
            "bq": np.ascontiguousarray(
                (br[0, hs, :].reshape(1, C) * scale).astype(bf)),
        })
    return in_maps


def _install_profile_hook():
    """Recreate the antenv.axon_hooks NTFF profile hook missing from this
    image (same ctypes ABI the axon boot script uses), and neuter the
    artifact upload which needs credentials we don't have."""
    if _CACHE.get("hook"):
        return
    import contextlib
    import ctypes
    import types

    mod = types.ModuleType("antenv.axon_hooks")
    _state = {}
    mod.set_axon_ntff_profile_hook = lambda h: _state.__setitem__("h", h)
    mod.get_axon_ntff_profile_hook = lambda: _state.get("h")
    sys.modules["antenv.axon_hooks"] = mod

    so_path = os.environ.get("PJRT_LIBRARY_PATH", "/opt/axon/libaxon_pjrt.so")
    lib = ctypes.CDLL(so_path)
    lib.axon_start_nrt_profile.argtypes = [
        ctypes.POINTER(ctypes.c_int64), ctypes.c_size_t]
    lib.axon_start_nrt_profile.restype = ctypes.c_int64
    lib.axon_stop_nrt_profile.argtypes = [ctypes.c_char_p]
    lib.axon_stop_nrt_profile.restype = ctypes.c_int64

    @contextlib.contextmanager
    def _hook(output_dir, device_ids):
        import jax
        jax.devices()
        if device_ids:
            ids = (ctypes.c_int64 * len(device_ids))(*device_ids)
            rc = lib.axon_start_nrt_profile(ids, len(device_ids))
        else:
            rc = lib.axon_start_nrt_profile(None, 0)
        if rc != 0:
            raise RuntimeError(f"axon_start_nrt_profile rc={rc}")
        try:
            yield
        finally:
            n = lib.axon_stop_nrt_profile(str(output_dir).encode())
            print(f"profile: {n} file(s) written to {output_dir}")

    mod.set_axon_ntff_profile_hook(_hook)

    from concourse import bass_utils as bu
    bu.upload_artifacts = lambda tmpdir: str(tmpdir)
    _CACHE["hook"] = True


def run(inputs, trace=False):
    if trace:
        _install_profile_hook()
    if "nc" not in _CACHE:
        _CACHE["nc"] = build_graph()
    nc = _CACHE["nc"]
    in_maps = _shard_inputs(
        inputs["x"], inputs["W_qkv"], inputs["b_qkv"], inputs["W_out"])
    res = run_bass_kernel_spmd(nc, in_maps, list(range(N_CORES)), trace=trace)
    acc = np.zeros((N, D), dtype=np.float32)
    for m in res.results:
        acc += np.asarray(m["out"], dtype=np.float32)
    # host-side constant terms: out bias + the v-bias pushed through the
    # out projection (softmax weights sum to 1, so it is a constant shift)
    bv = np.asarray(inputs["b_qkv"], dtype=np.float32).reshape(3, 16 * DH)[2]
    acc += np.asarray(inputs["b_out"], dtype=np.float32)[None, :]
    acc += (bv @ np.asarray(inputs["W_out"], dtype=np.float32))[None, :]
    return acc.reshape(1, N, D), res


def kernel(**inputs):
    out, _ = run(inputs, trace=False)
    return out


# revision 25
# speedup vs baseline: 1.0389x; 1.0389x over previous
"""Multi-head attention (b=1, n=2048, d_model=1024, 16 heads x 64) on 8 TRN2
NeuronCores, head-parallel tensor parallelism: each core computes 2 heads end
to end (qkv projection for its heads, attention, and its slice of the output
projection); the 8 partial outputs (rank-128 slices of the out-proj
contraction) are summed on the host along with b_out and the v-bias
correction (bv @ W_out, constant through softmax-normalized attention).

Device kernel per core (bf16 compute, f32 accumulation in PSUM):
  A) load x [2048,1024] f32, cast to bf16 on GPSIMD, XBAR DMA-transpose
     (SBUF->SBUF) -> xT; the PE never touches the transpose
  B) qT/kT = W^T x^T (q bias folded into the PSUM->SBUF copy via a
     per-partition tensor_scalar add; k bias dropped -- it cancels in
     softmax; v bias moved to host), v natural per tile with a ones
     column per head for the softmax row-sums
  C) per 512-query chunk: S^T = K Q^T per head (64-row contraction, no
     zero padding) -> exp on ACT -> P^T bf16; A^T_aug = V_aug^T P^T
     accumulated over key tiles in PSUM; softmax denominators ride in
     row 64 of the AV psum
  D) normalize with reciprocal_approx_fast + rank-1 PE broadcast (both
     emitted off the PE critical path), partial_out = A^T^T W_out -> f16
"""

import os
import sys

sys.path.insert(0, "/opt/trn_rl_repo")

import numpy as np
import ml_dtypes

import concourse.bass as bass
import concourse.tile as tile
from concourse import bacc, mybir
from concourse.bass_utils import run_bass_kernel_spmd
from concourse.masks import make_identity

F32 = mybir.dt.float32
F16 = mybir.dt.float16
BF16 = mybir.dt.bfloat16

N = 2048          # sequence length
D = 1024          # d_model
H_PER_CORE = 2    # heads per core
DH = 64           # head dim
C = H_PER_CORE * DH   # per-core qkv width = 128
N_CORES = 8
P = 128
N_TILES = N // P      # 16
D_TILES = D // P      # 8
I_CHUNK = 512         # query-chunk width for attention
N_ICHUNKS = N // I_CHUNK  # 4

_CACHE = {}


def build_graph():
    nc = bacc.Bacc()

    x_ext = nc.declare_dram_parameter("x", [N, D], F32, isOutput=False)
    wq_ext = nc.declare_dram_parameter("wq", [D, C], BF16, isOutput=False)
    wk_ext = nc.declare_dram_parameter("wk", [D, C], BF16, isOutput=False)
    wv_ext = nc.declare_dram_parameter("wv", [D, C], BF16, isOutput=False)
    wo_ext = nc.declare_dram_parameter("wo", [C, D], BF16, isOutput=False)
    bq_ext = nc.declare_dram_parameter("bq", [1, C], BF16, isOutput=False)
    out_ext = nc.declare_dram_parameter("out", [N, D], F16, isOutput=True)
    debug = bool(os.environ.get("KDEBUG"))
    if debug:
        qt_dbg = nc.declare_dram_parameter("qt_dbg", [P, N], BF16, isOutput=True)
        kt_dbg = nc.declare_dram_parameter("kt_dbg", [P, N], BF16, isOutput=True)
        v_dbg = nc.declare_dram_parameter(
            "v_dbg", [P, N_TILES * 2 * (DH + 1)], BF16, isOutput=True)
        pt_dbg = nc.declare_dram_parameter(
            "pt_dbg", [P, 2 * I_CHUNK], BF16, isOutput=True)
        at_dbg = nc.declare_dram_parameter("at_dbg", [P, N], BF16, isOutput=True)
        den_dbg = nc.declare_dram_parameter(
            "den_dbg", [1, 8 * I_CHUNK], F32, isOutput=True)
        rin_dbg = nc.declare_dram_parameter(
            "rin_dbg", [1, 8 * I_CHUNK], F32, isOutput=True)

    EXP = mybir.ActivationFunctionType.Exp
    MULT = mybir.AluOpType.mult

    with tile.TileContext(nc) as tc:
        with (
            tc.tile_pool(name="persist", bufs=1) as persist,
            tc.tile_pool(name="xload", bufs=10) as xload,
            tc.tile_pool(name="xcast", bufs=6) as xcast,
            tc.tile_pool(name="pt", bufs=33) as ptpool,
            tc.tile_pool(name="small", bufs=2) as small,
            tc.tile_pool(name="rbcp", bufs=2) as rbcp,
            tc.tile_pool(name="outsb", bufs=3) as outsb,
            tc.tile_pool(name="ps_s", bufs=2, space="PSUM") as ps_s,
            tc.tile_pool(name="ps_av", bufs=2, space="PSUM") as ps_av,
            tc.tile_pool(name="ps_mm", bufs=2, space="PSUM") as ps_mm,
        ):
            ones_col = persist.tile([1, P], BF16)
            nc.gpsimd.memset(ones_col, 1.0)
            ones_row = persist.tile([1, I_CHUNK], BF16)
            nc.gpsimd.memset(ones_row, 1.0)

            # --- weights / biases to SBUF ---
            wq_sb = persist.tile([P, D_TILES, C], BF16)
            wk_sb = persist.tile([P, D_TILES, C], BF16)
            wv_sb = persist.tile([P, D_TILES, C], BF16)
            wo_sb = persist.tile([C, D], BF16)
            bq_sb = persist.tile([1, C], BF16)

            # warm the ACT exp table before the first real activation
            warm = persist.tile([1, 1], BF16)
            nc.scalar.activation(warm[:], ones_col[0:1, 0:1], EXP)

            xT = persist.tile([P, D_TILES, N], BF16)
            qT = persist.tile([P, N], BF16)   # rows: h0 dims 0:64, h1 64:128
            kT = persist.tile([P, N], BF16)   # same packing, no zero pad
            # per key-tile: [v_h0 (64) | 1 | v_h1 (64) | 1]
            v_sb = persist.tile([P, N_TILES, 2 * (DH + 1)], BF16)
            nc.gpsimd.memset(v_sb[:, :, DH:DH + 1], 1.0)
            nc.gpsimd.memset(v_sb[:, :, 2 * DH + 1:2 * DH + 2], 1.0)
            aT = persist.tile([P, N], BF16)   # normalized A^T, heads stacked

            pts = {c: [] for c in range(N_ICHUNKS)}
            osb_ref = {}
            if debug:
                dendbg_sb = persist.tile([1, 8 * I_CHUNK], F32)
                rindbg_sb = persist.tile([1, 8 * I_CHUNK], F32)

            def emit_score(c, j):
                """S^T tile for both heads of (chunk c, key tile j) + exp."""
                sps = ps_s.tile([P, 2 * I_CHUNK], F32, tag="s", name=f"s{c}_{j}")
                ccols = slice(c * I_CHUNK, (c + 1) * I_CHUNK)
                jcols = slice(j * P, (j + 1) * P)
                for h in range(H_PER_CORE):
                    hr = slice(h * DH, (h + 1) * DH)
                    nc.tensor.matmul(
                        sps[:, h * I_CHUNK:(h + 1) * I_CHUNK],
                        kT[hr, jcols], qT[hr, ccols], start=True, stop=True)
                pt = ptpool.tile([P, 2 * I_CHUNK], BF16, tag="pt",
                                 name=f"pt{c}_{j}")
                nc.scalar.activation(pt[:], sps[:], EXP)
                pts[c].append(pt)

            def emit_av_step(avps, c, j):
                for h in range(H_PER_CORE):
                    nc.tensor.matmul(
                        avps[h][:],
                        v_sb[:, j, h * (DH + 1):(h + 1) * (DH + 1)],
                        pts[c][j][:, h * I_CHUNK:(h + 1) * I_CHUNK],
                        start=(j == 0), stop=(j == N_TILES - 1))

            def emit_norm_recip(c, avps):
                """DVE: pull both heads' denominator rows (psum partition 64)
                to a partition-0 SBUF row, then one fast reciprocal + cast.
                reciprocal_approx_fast silently ignores a psum partition
                offset, so the copy is required for correctness."""
                den = small.tile([1, 2 * I_CHUNK], F32, tag="den",
                                 name=f"den{c}")
                for h in range(H_PER_CORE):
                    nc.vector.tensor_copy(
                        out=den[:, h * I_CHUNK:(h + 1) * I_CHUNK],
                        in_=avps[h][DH:DH + 1, :])
                rinv = small.tile([1, 2 * I_CHUNK], F32, tag="rinv",
                                  name=f"rinv{c}")
                nc.vector.reciprocal_approx_fast(rinv[:], den[:])
                rbf = small.tile([1, 2 * I_CHUNK], BF16, tag="rbf",
                                 name=f"rbf{c}")
                nc.vector.tensor_copy(out=rbf[:], in_=rinv[:])
                if debug:
                    i0 = 2 * c * I_CHUNK
                    nc.vector.tensor_copy(
                        out=dendbg_sb[:, i0:i0 + 2 * I_CHUNK], in_=den[:])
                    nc.vector.tensor_copy(
                        out=rindbg_sb[:, i0:i0 + 2 * I_CHUNK], in_=rinv[:])
                return rbf

            def emit_norm_mult(c, h, avps, rbf):
                """PE rank-1 broadcast of 1/denom, then DVE row scale -> aT."""
                rbc_ps = ps_mm.tile([P, I_CHUNK], F32, tag="mm",
                                    name=f"rbc{c}_{h}")
                nc.tensor.matmul(
                    rbc_ps[:], ones_col[:],
                    rbf[:, h * I_CHUNK:(h + 1) * I_CHUNK],
                    start=True, stop=True)
                rbc_sb = rbcp.tile([P, I_CHUNK], F32, tag="rbc",
                                   name=f"rbcs{c}_{h}")
                nc.vector.tensor_copy(out=rbc_sb[:], in_=rbc_ps[:])
                ccols = slice(c * I_CHUNK, (c + 1) * I_CHUNK)
                nc.vector.tensor_tensor(
                    aT[h * DH:(h + 1) * DH, ccols], avps[h][0:DH, :],
                    rbc_sb[0:DH, :], MULT)

            def emit_outproj_step(c, k):
                """One of 8 out-proj matmuls for chunk c (4 q-tiles x 2),
                with its own f16 staging half and immediate DMA out."""
                iblk = c * 4 + k // 2
                nn = k % 2
                ops = ps_mm.tile([P, I_CHUNK], F32, tag="mm",
                                 name=f"op{c}_{k}")
                nc.tensor.matmul(
                    ops[:], aT[:, iblk * P:(iblk + 1) * P],
                    wo_sb[:, nn * I_CHUNK:(nn + 1) * I_CHUNK],
                    start=True, stop=True)
                osb = outsb.tile([P, I_CHUNK], F16, tag="osb",
                                 name=f"osb{c}_{k}")
                nc.vector.tensor_copy(out=osb[:], in_=ops[:])
                dma_eng = (nc.sync, nc.gpsimd)[(iblk * 2 + nn) % 2]
                dma_eng.dma_start(
                    out_ext[iblk * P:(iblk + 1) * P,
                            nn * I_CHUNK:(nn + 1) * I_CHUNK], osb[:])

            # --- first x tiles in flight, then weights on the sync queue
            # (first-use order, wo last), then the per-group pipeline.
            xf_tiles = {}

            def issue_xload(t):
                xf = xload.tile([P, D], F32, tag="xf", name=f"xf{t}")
                eng = (nc.sync, nc.gpsimd)[t % 2]
                eng.dma_start(xf[:], x_ext[t * P:(t + 1) * P, :])
                xf_tiles[t] = xf

            issue_xload(0)
            issue_xload(1)
            issue_xload(2)
            issue_xload(3)
            nc.sync.dma_start(
                wv_sb[:], wv_ext[:].rearrange("(o p) c -> p o c", p=P))
            nc.sync.dma_start(
                wk_sb[:], wk_ext[:].rearrange("(o p) c -> p o c", p=P))
            nc.sync.dma_start(
                wq_sb[:], wq_ext[:].rearrange("(o p) c -> p o c", p=P))
            nc.sync.dma_start(bq_sb[:], bq_ext[:])
            nc.sync.dma_start(wo_sb[:], wo_ext[:])

            # --- phase A+B (v4 structure): per group of 4 x-tiles: load +
            # DVE cast (one per group on GPSIMD) + XBAR transpose on sync,
            # then q/k projections, per-tile v-proj, chunk-0 scores.
            for g in range(N // I_CHUNK):
                for t in range(4 * g, 4 * g + 4):
                    if t not in xf_tiles:
                        issue_xload(t)
                    xb = xcast.tile([P, D], BF16, tag="xb", name=f"xb{t}")
                    cast_eng = nc.gpsimd if t % 4 == 3 else nc.vector
                    cast_eng.tensor_copy(out=xb[:], in_=xf_tiles[t][:])
                    nc.sync.dma_start_transpose(
                        xT[:, :, t * P:(t + 1) * P], xb[:])
                cols = slice(g * I_CHUNK, (g + 1) * I_CHUNK)
                psq = ps_mm.tile([P, I_CHUNK], F32, tag="mm", name=f"psq{g}")
                for do in range(D_TILES):
                    nc.tensor.matmul(
                        psq[:], wq_sb[:, do, :], xT[:, do, cols],
                        start=(do == 0), stop=False)
                nc.tensor.matmul(
                    psq[:], bq_sb[:], ones_row[:], start=False, stop=True)
                nc.vector.tensor_copy(out=qT[:, cols], in_=psq[:])
                psk = ps_mm.tile([P, I_CHUNK], F32, tag="mm", name=f"psk{g}")
                for do in range(D_TILES):
                    nc.tensor.matmul(
                        psk[:], wk_sb[:, do, :], xT[:, do, cols],
                        start=(do == 0), stop=(do == D_TILES - 1))
                nc.vector.tensor_copy(out=kT[:, cols], in_=psk[:])
                for t in range(4 * g, 4 * g + 4):
                    psv = ps_mm.tile([P, C], F32, tag="mm", name=f"psv{t}")
                    for do in range(D_TILES):
                        nc.tensor.matmul(
                            psv[:], xT[:, do, t * P:(t + 1) * P],
                            wv_sb[:, do, :],
                            start=(do == 0), stop=(do == D_TILES - 1))
                    nc.vector.tensor_copy(
                        out=v_sb[:, t, :].rearrange(
                            "p (h x) -> p h x", h=2)[:, :, 0:DH],
                        in_=psv[:].rearrange("p (h x) -> p h x", h=2))
                for j in range(4 * g, 4 * g + 4):
                    emit_score(0, j)

            # --- attention windows w=1..3: scores(w) + AV(w-1), with the
            # norm of chunk w-2 and its out-proj slotted in where their
            # inputs are ready.
            avps_saved = {}
            rbf_saved = {}
            for w in range(1, N_ICHUNKS):
                avps = [ps_av.tile([DH + 1, I_CHUNK], F32, tag="av",
                                   name=f"av{w - 1}_{h}")
                        for h in range(H_PER_CORE)]
                for j in range(0, 3):
                    emit_score(w, j)
                if w >= 2:
                    for h in range(H_PER_CORE):
                        emit_norm_mult(w - 2, h, avps_saved[w - 2],
                                       rbf_saved[w - 2])
                for j in range(3, 6):
                    emit_score(w, j)
                k_op = 0
                for j in range(6, N_TILES):
                    emit_score(w, j)
                    emit_av_step(avps, w - 1, j - 6)
                    if w >= 2 and k_op < 8:
                        emit_outproj_step(w - 2, k_op)
                        k_op += 1
                for j in range(N_TILES - 6, N_TILES):
                    emit_av_step(avps, w - 1, j)
                rbf_saved[w - 1] = emit_norm_recip(w - 1, avps)
                avps_saved[w - 1] = avps

            # --- final: norm+outproj(2), AV(3), norm(3), outproj(3)
            c2 = N_ICHUNKS - 2
            c3 = N_ICHUNKS - 1
            for h in range(H_PER_CORE):
                emit_norm_mult(c2, h, avps_saved[c2], rbf_saved[c2])
            avps3 = [ps_av.tile([DH + 1, I_CHUNK], F32, tag="av",
                                name=f"av{c3}_{h}")
                     for h in range(H_PER_CORE)]
            k_op = 0
            for j in range(N_TILES):
                emit_av_step(avps3, c3, j)
                if j >= 2 and k_op < 8:
                    emit_outproj_step(c2, k_op)
                    k_op += 1
            rbf3 = emit_norm_recip(c3, avps3)
            for h in range(H_PER_CORE):
                emit_norm_mult(c3, h, avps3, rbf3)
            for k in range(8):
                emit_outproj_step(c3, k)
            if debug:
                nc.sync.dma_start(qt_dbg[:], qT[:])
                nc.sync.dma_start(kt_dbg[:], kT[:])
                nc.sync.dma_start(
                    v_dbg[:], v_sb[:].rearrange("p t c -> p (t c)"))
                nc.sync.dma_start(pt_dbg[:], pts[0][0][:])
                nc.sync.dma_start(at_dbg[:], aT[:])
                nc.sync.dma_start(den_dbg[:], dendbg_sb[:])
                nc.sync.dma_start(rin_dbg[:], rindbg_sb[:])
    nc.compile()
    return nc


def _shard_inputs(x, W_qkv, b_qkv, W_out):
    x2d = np.ascontiguousarray(x.reshape(N, D), dtype=np.float32)
    Wr = np.asarray(W_qkv, dtype=np.float32).reshape(D, 3, 16, DH)
    br = np.asarray(b_qkv, dtype=np.float32).reshape(3, 16, DH)
    Wo = np.asarray(W_out, dtype=np.float32)
    scale = 1.0 / np.sqrt(DH)
    bf = ml_dtypes.bfloat16
    in_maps = []
    for c in range(N_CORES):
        hs = slice(2 * c, 2 * c + 2)
        in_maps.append({
            "x": x2d,
            "wq": np.ascontiguousarray(
                (Wr[:, 0, hs, :].reshape(D, C) * scale).astype(bf)),
            "wk": np.ascontiguousarray(Wr[:, 1, hs, :].reshape(D, C).astype(bf)),
            "wv": np.ascontiguousarray(Wr[:, 2, hs, :].reshape(D, C).astype(bf)),
            "wo": np.ascontiguousarray(Wo[c * C:(c + 1) * C, :].astype(bf)),
            "bq": np.ascontiguousarray(
                (br[0, hs, :].reshape(1, C) * scale).astype(bf)),
        })
    return in_maps


def _install_profile_hook():
    """Recreate the antenv.axon_hooks NTFF profile hook missing from this
    image (same ctypes ABI the axon boot script uses), and neuter the
    artifact upload which needs credentials we don't have."""
    if _CACHE.get("hook"):
        return
    import contextlib
    import ctypes
    import types

    mod = types.ModuleType("antenv.axon_hooks")
    _state = {}
    mod.set_axon_ntff_profile_hook = lambda h: _state.__setitem__("h", h)
    mod.get_axon_ntff_profile_hook = lambda: _state.get("h")
    sys.modules["antenv.axon_hooks"] = mod

    so_path = os.environ.get("PJRT_LIBRARY_PATH", "/opt/axon/libaxon_pjrt.so")
    lib = ctypes.CDLL(so_path)
    lib.axon_start_nrt_profile.argtypes = [
        ctypes.POINTER(ctypes.c_int64), ctypes.c_size_t]
    lib.axon_start_nrt_profile.restype = ctypes.c_int64
    lib.axon_stop_nrt_profile.argtypes = [ctypes.c_char_p]
    lib.axon_stop_nrt_profile.restype = ctypes.c_int64

    @contextlib.contextmanager
    def _hook(output_dir, device_ids):
        import jax
        jax.devices()
        if device_ids:
            ids = (ctypes.c_int64 * len(device_ids))(*device_ids)
            rc = lib.axon_start_nrt_profile(ids, len(device_ids))
        else:
            rc = lib.axon_start_nrt_profile(None, 0)
        if rc != 0:
            raise RuntimeError(f"axon_start_nrt_profile rc={rc}")
        try:
            yield
        finally:
            n = lib.axon_stop_nrt_profile(str(output_dir).encode())
            print(f"profile: {n} file(s) written to {output_dir}")

    mod.set_axon_ntff_profile_hook(_hook)

    from concourse import bass_utils as bu
    bu.upload_artifacts = lambda tmpdir: str(tmpdir)
    _CACHE["hook"] = True


def run(inputs, trace=False):
    if trace:
        _install_profile_hook()
    if "nc" not in _CACHE:
        _CACHE["nc"] = build_graph()
    nc = _CACHE["nc"]
    in_maps = _shard_inputs(
        inputs["x"], inputs["W_qkv"], inputs["b_qkv"], inputs["W_out"])
    res = run_bass_kernel_spmd(nc, in_maps, list(range(N_CORES)), trace=trace)
    acc = np.zeros((N, D), dtype=np.float32)
    for m in res.results:
        acc += np.asarray(m["out"], dtype=np.float32)
    # host-side constant terms: out bias + the v-bias pushed through the
    # out projection (softmax weights sum to 1, so it is a constant shift)
    bv = np.asarray(inputs["b_qkv"], dtype=np.float32).reshape(3, 16 * DH)[2]
    acc += np.asarray(inputs["b_out"], dtype=np.float32)[None, :]
    acc += (bv @ np.asarray(inputs["W_out"], dtype=np.float32))[None, :]
    return acc.reshape(1, N, D), res


def kernel(**inputs):
    out, _ = run(inputs, trace=False)
    return out


# revision 26
# speedup vs baseline: 1.2231x; 1.1773x over previous
"""Multi-head attention (b=1, n=2048, d_model=1024, 16 heads x 64) on 8 TRN2
NeuronCores, head-parallel tensor parallelism: each core computes 2 heads end
to end (qkv projection for its heads, attention, and its slice of the output
projection); the 8 partial outputs (rank-128 slices of the out-proj
contraction) are summed on the host along with b_out and the v-bias
correction (bv @ W_out, constant through softmax-normalized attention).

Device kernel per core (bf16 compute, f32 accumulation in PSUM):
  A) load x [2048,1024] f32, cast to bf16 on GPSIMD, XBAR DMA-transpose
     (SBUF->SBUF) -> xT; the PE never touches the transpose
  B) qT/kT = W^T x^T (q bias folded into the PSUM->SBUF copy via a
     per-partition tensor_scalar add; k bias dropped -- it cancels in
     softmax; v bias moved to host), v natural per tile with a ones
     column per head for the softmax row-sums
  C) per 512-query chunk: S^T = K Q^T per head (64-row contraction, no
     zero padding) -> exp on ACT -> P^T bf16; A^T_aug = V_aug^T P^T
     accumulated over key tiles in PSUM; softmax denominators ride in
     row 64 of the AV psum
  D) normalize with reciprocal_approx_fast + rank-1 PE broadcast (both
     emitted off the PE critical path), partial_out = A^T^T W_out -> f16
"""

import os
import sys

sys.path.insert(0, "/opt/trn_rl_repo")

import numpy as np
import ml_dtypes

import concourse.bass as bass
import concourse.tile as tile
from concourse import bacc, mybir
from concourse.bass_utils import run_bass_kernel_spmd
from concourse.masks import make_identity

F32 = mybir.dt.float32
F16 = mybir.dt.float16
BF16 = mybir.dt.bfloat16

N = 2048          # sequence length
D = 1024          # d_model
H_PER_CORE = 2    # heads per core
DH = 64           # head dim
C = H_PER_CORE * DH   # per-core qkv width = 128
N_CORES = 8
P = 128
N_TILES = N // P      # 16
D_TILES = D // P      # 8
I_CHUNK = 512         # query-chunk width for attention
N_ICHUNKS = N // I_CHUNK  # 4

_CACHE = {}


def build_graph():
    nc = bacc.Bacc()

    x_ext = nc.declare_dram_parameter("x", [N, D], F32, isOutput=False)
    wq_ext = nc.declare_dram_parameter("wq", [D, C], BF16, isOutput=False)
    wk_ext = nc.declare_dram_parameter("wk", [D, C], BF16, isOutput=False)
    wv_ext = nc.declare_dram_parameter("wv", [D, C], BF16, isOutput=False)
    wo_ext = nc.declare_dram_parameter("wo", [C, D], BF16, isOutput=False)
    bq_ext = nc.declare_dram_parameter("bq", [1, C], BF16, isOutput=False)
    out_ext = nc.declare_dram_parameter("out", [N, D], F16, isOutput=True)
    debug = bool(os.environ.get("KDEBUG"))
    if debug:
        qt_dbg = nc.declare_dram_parameter("qt_dbg", [P, N], BF16, isOutput=True)
        kt_dbg = nc.declare_dram_parameter("kt_dbg", [P, N], BF16, isOutput=True)
        v_dbg = nc.declare_dram_parameter(
            "v_dbg", [P, N_TILES * 2 * (DH + 1)], BF16, isOutput=True)
        pt_dbg = nc.declare_dram_parameter(
            "pt_dbg", [P, 2 * I_CHUNK], BF16, isOutput=True)
        at_dbg = nc.declare_dram_parameter("at_dbg", [P, N], BF16, isOutput=True)
        den_dbg = nc.declare_dram_parameter(
            "den_dbg", [1, 8 * I_CHUNK], F32, isOutput=True)
        rin_dbg = nc.declare_dram_parameter(
            "rin_dbg", [1, 8 * I_CHUNK], F32, isOutput=True)

    EXP = mybir.ActivationFunctionType.Exp
    MULT = mybir.AluOpType.mult

    with tile.TileContext(nc) as tc:
        with (
            tc.tile_pool(name="persist", bufs=1) as persist,
            tc.tile_pool(name="xload", bufs=10) as xload,
            tc.tile_pool(name="xcast", bufs=6) as xcast,
            tc.tile_pool(name="pt", bufs=33) as ptpool,
            tc.tile_pool(name="small", bufs=2) as small,
            tc.tile_pool(name="rbcp", bufs=2) as rbcp,
            tc.tile_pool(name="outsb", bufs=3) as outsb,
            tc.tile_pool(name="ps_s", bufs=2, space="PSUM") as ps_s,
            tc.tile_pool(name="ps_av", bufs=2, space="PSUM") as ps_av,
            tc.tile_pool(name="ps_mm", bufs=2, space="PSUM") as ps_mm,
        ):
            ident = persist.tile([P, P], BF16)
            make_identity(nc, ident)
            ones_col = persist.tile([1, P], BF16)
            nc.gpsimd.memset(ones_col, 1.0)
            ones_row = persist.tile([1, I_CHUNK], BF16)
            nc.gpsimd.memset(ones_row, 1.0)

            # --- weights / biases to SBUF ---
            wq_sb = persist.tile([P, D_TILES, C], BF16)
            wk_sb = persist.tile([P, D_TILES, C], BF16)
            wv_sb = persist.tile([P, D_TILES, C], BF16)
            wo_sb = persist.tile([C, D], BF16)
            bq_sb = persist.tile([1, C], BF16)

            # warm the ACT exp table before the first real activation
            warm = persist.tile([1, 1], BF16)
            nc.scalar.activation(warm[:], ones_col[0:1, 0:1], EXP)

            xT = persist.tile([P, D_TILES, N], BF16)
            qT = persist.tile([P, N], BF16)   # rows: h0 dims 0:64, h1 64:128
            kT = persist.tile([P, N], BF16)   # same packing, no zero pad
            # per key-tile: [v_h0 (64) | 1 | v_h1 (64) | 1]
            v_sb = persist.tile([P, N_TILES, 2 * (DH + 1)], BF16)
            nc.gpsimd.memset(v_sb[:, :, DH:DH + 1], 1.0)
            nc.gpsimd.memset(v_sb[:, :, 2 * DH + 1:2 * DH + 2], 1.0)
            aT = persist.tile([P, N], BF16)   # normalized A^T, heads stacked

            pts = {c: [] for c in range(N_ICHUNKS)}
            osb_ref = {}
            if debug:
                dendbg_sb = persist.tile([1, 8 * I_CHUNK], F32)
                rindbg_sb = persist.tile([1, 8 * I_CHUNK], F32)

            def emit_score(c, j):
                """S^T tile for both heads of (chunk c, key tile j) + exp."""
                sps = ps_s.tile([P, 2 * I_CHUNK], F32, tag="s", name=f"s{c}_{j}")
                ccols = slice(c * I_CHUNK, (c + 1) * I_CHUNK)
                jcols = slice(j * P, (j + 1) * P)
                for h in range(H_PER_CORE):
                    hr = slice(h * DH, (h + 1) * DH)
                    nc.tensor.matmul(
                        sps[:, h * I_CHUNK:(h + 1) * I_CHUNK],
                        kT[hr, jcols], qT[hr, ccols], start=True, stop=True)
                pt = ptpool.tile([P, 2 * I_CHUNK], BF16, tag="pt",
                                 name=f"pt{c}_{j}")
                nc.scalar.activation(pt[:], sps[:], EXP)
                pts[c].append(pt)

            def emit_av_step(avps, c, j):
                for h in range(H_PER_CORE):
                    nc.tensor.matmul(
                        avps[h][:],
                        v_sb[:, j, h * (DH + 1):(h + 1) * (DH + 1)],
                        pts[c][j][:, h * I_CHUNK:(h + 1) * I_CHUNK],
                        start=(j == 0), stop=(j == N_TILES - 1))

            def emit_norm_recip(c, avps):
                """DVE: pull both heads' denominator rows (psum partition 64)
                to a partition-0 SBUF row, then one fast reciprocal + cast.
                reciprocal_approx_fast silently ignores a psum partition
                offset, so the copy is required for correctness."""
                den = small.tile([1, 2 * I_CHUNK], F32, tag="den",
                                 name=f"den{c}")
                for h in range(H_PER_CORE):
                    nc.vector.tensor_copy(
                        out=den[:, h * I_CHUNK:(h + 1) * I_CHUNK],
                        in_=avps[h][DH:DH + 1, :])
                rinv = small.tile([1, 2 * I_CHUNK], F32, tag="rinv",
                                  name=f"rinv{c}")
                nc.vector.reciprocal_approx_fast(rinv[:], den[:])
                rbf = small.tile([1, 2 * I_CHUNK], BF16, tag="rbf",
                                 name=f"rbf{c}")
                nc.vector.tensor_copy(out=rbf[:], in_=rinv[:])
                if debug:
                    i0 = 2 * c * I_CHUNK
                    nc.vector.tensor_copy(
                        out=dendbg_sb[:, i0:i0 + 2 * I_CHUNK], in_=den[:])
                    nc.vector.tensor_copy(
                        out=rindbg_sb[:, i0:i0 + 2 * I_CHUNK], in_=rinv[:])
                return rbf

            def emit_norm_mult(c, h, avps, rbf):
                """PE rank-1 broadcast of 1/denom, then DVE row scale -> aT."""
                rbc_ps = ps_mm.tile([P, I_CHUNK], F32, tag="mm",
                                    name=f"rbc{c}_{h}")
                nc.tensor.matmul(
                    rbc_ps[:], ones_col[:],
                    rbf[:, h * I_CHUNK:(h + 1) * I_CHUNK],
                    start=True, stop=True)
                rbc_sb = rbcp.tile([P, I_CHUNK], F32, tag="rbc",
                                   name=f"rbcs{c}_{h}")
                nc.vector.tensor_copy(out=rbc_sb[:], in_=rbc_ps[:])
                ccols = slice(c * I_CHUNK, (c + 1) * I_CHUNK)
                nc.vector.tensor_tensor(
                    aT[h * DH:(h + 1) * DH, ccols], avps[h][0:DH, :],
                    rbc_sb[0:DH, :], MULT)

            def emit_outproj_step(c, k):
                """One of 8 out-proj matmuls for chunk c (4 q-tiles x 2),
                with its own f16 staging half and immediate DMA out."""
                iblk = c * 4 + k // 2
                nn = k % 2
                ops = ps_mm.tile([P, I_CHUNK], F32, tag="mm",
                                 name=f"op{c}_{k}")
                nc.tensor.matmul(
                    ops[:], aT[:, iblk * P:(iblk + 1) * P],
                    wo_sb[:, nn * I_CHUNK:(nn + 1) * I_CHUNK],
                    start=True, stop=True)
                osb = outsb.tile([P, I_CHUNK], F16, tag="osb",
                                 name=f"osb{c}_{k}")
                nc.vector.tensor_copy(out=osb[:], in_=ops[:])
                dma_eng = (nc.sync, nc.gpsimd)[(iblk * 2 + nn) % 2]
                dma_eng.dma_start(
                    out_ext[iblk * P:(iblk + 1) * P,
                            nn * I_CHUNK:(nn + 1) * I_CHUNK], osb[:])

            # --- first x tiles in flight, then weights on the sync queue
            # (first-use order, wo last), then the per-group pipeline.
            xf_tiles = {}

            def issue_xload(t):
                xf = xload.tile([P, D], F32, tag="xf", name=f"xf{t}")
                eng = (nc.sync, nc.gpsimd)[t % 2]
                eng.dma_start(xf[:], x_ext[t * P:(t + 1) * P, :])
                xf_tiles[t] = xf

            issue_xload(0)
            issue_xload(1)
            issue_xload(2)
            issue_xload(3)
            nc.sync.dma_start(
                wv_sb[:], wv_ext[:].rearrange("(o p) c -> p o c", p=P))
            nc.sync.dma_start(
                wk_sb[:], wk_ext[:].rearrange("(o p) c -> p o c", p=P))
            nc.sync.dma_start(
                wq_sb[:], wq_ext[:].rearrange("(o p) c -> p o c", p=P))
            nc.sync.dma_start(bq_sb[:], bq_ext[:])
            nc.sync.dma_start(wo_sb[:], wo_ext[:])

            # --- phase A+B (v4 structure): per group of 4 x-tiles: load +
            # DVE cast (one per group on GPSIMD) + XBAR transpose on sync,
            # then q/k projections, per-tile v-proj, chunk-0 scores.
            for g in range(N // I_CHUNK):
                xbs = {}
                for t in range(4 * g, 4 * g + 4):
                    if t not in xf_tiles:
                        issue_xload(t)
                    xb = xcast.tile([P, D], BF16, tag="xb", name=f"xb{t}")
                    nc.vector.tensor_copy(out=xb[:], in_=xf_tiles[t][:])
                    xbs[t] = xb
                for t in range(4 * g, 4 * g + 4):
                    tp = ps_mm.tile([P, D_TILES, P], BF16, tag="mm",
                                    name=f"tp{t}")
                    for do in range(D_TILES):
                        nc.tensor.transpose(
                            tp[:, do, :], xbs[t][:, do * P:(do + 1) * P],
                            ident)
                    drain_eng = nc.vector if t % 2 == 0 else None
                    if drain_eng is None:
                        nc.scalar.activation(
                            xT[:, :, t * P:(t + 1) * P], tp[:],
                            mybir.ActivationFunctionType.Copy)
                    else:
                        drain_eng.tensor_copy(
                            out=xT[:, :, t * P:(t + 1) * P], in_=tp[:])
                cols = slice(g * I_CHUNK, (g + 1) * I_CHUNK)
                psq = ps_mm.tile([P, I_CHUNK], F32, tag="mm", name=f"psq{g}")
                for do in range(D_TILES):
                    nc.tensor.matmul(
                        psq[:], wq_sb[:, do, :], xT[:, do, cols],
                        start=(do == 0), stop=False)
                nc.tensor.matmul(
                    psq[:], bq_sb[:], ones_row[:], start=False, stop=True)
                nc.vector.tensor_copy(out=qT[:, cols], in_=psq[:])
                psk = ps_mm.tile([P, I_CHUNK], F32, tag="mm", name=f"psk{g}")
                for do in range(D_TILES):
                    nc.tensor.matmul(
                        psk[:], wk_sb[:, do, :], xT[:, do, cols],
                        start=(do == 0), stop=(do == D_TILES - 1))
                nc.vector.tensor_copy(out=kT[:, cols], in_=psk[:])
                for t in range(4 * g, 4 * g + 4):
                    psv = ps_mm.tile([P, C], F32, tag="mm", name=f"psv{t}")
                    for do in range(D_TILES):
                        nc.tensor.matmul(
                            psv[:], xT[:, do, t * P:(t + 1) * P],
                            wv_sb[:, do, :],
                            start=(do == 0), stop=(do == D_TILES - 1))
                    nc.vector.tensor_copy(
                        out=v_sb[:, t, :].rearrange(
                            "p (h x) -> p h x", h=2)[:, :, 0:DH],
                        in_=psv[:].rearrange("p (h x) -> p h x", h=2))
                for j in range(4 * g, 4 * g + 4):
                    emit_score(0, j)

            # --- attention windows w=1..3: scores(w) + AV(w-1), with the
            # norm of chunk w-2 and its out-proj slotted in where their
            # inputs are ready.
            avps_saved = {}
            rbf_saved = {}
            for w in range(1, N_ICHUNKS):
                avps = [ps_av.tile([DH + 1, I_CHUNK], F32, tag="av",
                                   name=f"av{w - 1}_{h}")
                        for h in range(H_PER_CORE)]
                for j in range(0, 3):
                    emit_score(w, j)
                if w >= 2:
                    for h in range(H_PER_CORE):
                        emit_norm_mult(w - 2, h, avps_saved[w - 2],
                                       rbf_saved[w - 2])
                for j in range(3, 6):
                    emit_score(w, j)
                k_op = 0
                for j in range(6, N_TILES):
                    emit_score(w, j)
                    emit_av_step(avps, w - 1, j - 6)
                    if w >= 2 and k_op < 8:
                        emit_outproj_step(w - 2, k_op)
                        k_op += 1
                for j in range(N_TILES - 6, N_TILES):
                    emit_av_step(avps, w - 1, j)
                rbf_saved[w - 1] = emit_norm_recip(w - 1, avps)
                avps_saved[w - 1] = avps

            # --- final: norm+outproj(2), AV(3), norm(3), outproj(3)
            c2 = N_ICHUNKS - 2
            c3 = N_ICHUNKS - 1
            for h in range(H_PER_CORE):
                emit_norm_mult(c2, h, avps_saved[c2], rbf_saved[c2])
            avps3 = [ps_av.tile([DH + 1, I_CHUNK], F32, tag="av",
                                name=f"av{c3}_{h}")
                     for h in range(H_PER_CORE)]
            k_op = 0
            for j in range(N_TILES):
                emit_av_step(avps3, c3, j)
                if j >= 2 and k_op < 8:
                    emit_outproj_step(c2, k_op)
                    k_op += 1
            rbf3 = emit_norm_recip(c3, avps3)
            for h in range(H_PER_CORE):
                emit_norm_mult(c3, h, avps3, rbf3)
            for k in range(8):
                emit_outproj_step(c3, k)
            if debug:
                nc.sync.dma_start(qt_dbg[:], qT[:])
                nc.sync.dma_start(kt_dbg[:], kT[:])
                nc.sync.dma_start(
                    v_dbg[:], v_sb[:].rearrange("p t c -> p (t c)"))
                nc.sync.dma_start(pt_dbg[:], pts[0][0][:])
                nc.sync.dma_start(at_dbg[:], aT[:])
                nc.sync.dma_start(den_dbg[:], dendbg_sb[:])
                nc.sync.dma_start(rin_dbg[:], rindbg_sb[:])
    nc.compile()
    return nc


def _shard_inputs(x, W_qkv, b_qkv, W_out):
    x2d = np.ascontiguousarray(x.reshape(N, D), dtype=np.float32)
    Wr = np.asarray(W_qkv, dtype=np.float32).reshape(D, 3, 16, DH)
    br = np.asarray(b_qkv, dtype=np.float32).reshape(3, 16, DH)
    Wo = np.asarray(W_out, dtype=np.float32)
    scale = 1.0 / np.sqrt(DH)
    bf = ml_dtypes.bfloat16
    in_maps = []
    for c in range(N_CORES):
        hs = slice(2 * c, 2 * c + 2)
        in_maps.append({
            "x": x2d,
            "wq": np.ascontiguousarray(
                (Wr[:, 0, hs, :].reshape(D, C) * scale).astype(bf)),
            "wk": np.ascontiguousarray(Wr[:, 1, hs, :].reshape(D, C).astype(bf)),
            "wv": np.ascontiguousarray(Wr[:, 2, hs, :].reshape(D, C).astype(bf)),
            "wo": np.ascontiguousarray(Wo[c * C:(c + 1) * C, :].astype(bf)),
            "bq": np.ascontiguousarray(
                (br[0, hs, :].reshape(1, C) * scale).astype(bf)),
        })
    return in_maps


def _install_profile_hook():
    """Recreate the antenv.axon_hooks NTFF profile hook missing from this
    image (same ctypes ABI the axon boot script uses), and neuter the
    artifact upload which needs credentials we don't have."""
    if _CACHE.get("hook"):
        return
    import contextlib
    import ctypes
    import types

    mod = types.ModuleType("antenv.axon_hooks")
    _state = {}
    mod.set_axon_ntff_profile_hook = lambda h: _state.__setitem__("h", h)
    mod.get_axon_ntff_profile_hook = lambda: _state.get("h")
    sys.modules["antenv.axon_hooks"] = mod

    so_path = os.environ.get("PJRT_LIBRARY_PATH", "/opt/axon/libaxon_pjrt.so")
    lib = ctypes.CDLL(so_path)
    lib.axon_start_nrt_profile.argtypes = [
        ctypes.POINTER(ctypes.c_int64), ctypes.c_size_t]
    lib.axon_start_nrt_profile.restype = ctypes.c_int64
    lib.axon_stop_nrt_profile.argtypes = [ctypes.c_char_p]
    lib.axon_stop_nrt_profile.restype = ctypes.c_int64

    @contextlib.contextmanager
    def _hook(output_dir, device_ids):
        import jax
        jax.devices()
        if device_ids:
            ids = (ctypes.c_int64 * len(device_ids))(*device_ids)
            rc = lib.axon_start_nrt_profile(ids, len(device_ids))
        else:
            rc = lib.axon_start_nrt_profile(None, 0)
        if rc != 0:
            raise RuntimeError(f"axon_start_nrt_profile rc={rc}")
        try:
            yield
        finally:
            n = lib.axon_stop_nrt_profile(str(output_dir).encode())
            print(f"profile: {n} file(s) written to {output_dir}")

    mod.set_axon_ntff_profile_hook(_hook)

    from concourse import bass_utils as bu
    bu.upload_artifacts = lambda tmpdir: str(tmpdir)
    _CACHE["hook"] = True


def run(inputs, trace=False):
    if trace:
        _install_profile_hook()
    if "nc" not in _CACHE:
        _CACHE["nc"] = build_graph()
    nc = _CACHE["nc"]
    in_maps = _shard_inputs(
        inputs["x"], inputs["W_qkv"], inputs["b_qkv"], inputs["W_out"])
    res = run_bass_kernel_spmd(nc, in_maps, list(range(N_CORES)), trace=trace)
    acc = np.zeros((N, D), dtype=np.float32)
    for m in res.results:
        acc += np.asarray(m["out"], dtype=np.float32)
    # host-side constant terms: out bias + the v-bias pushed through the
    # out projection (softmax weights sum to 1, so it is a constant shift)
    bv = np.asarray(inputs["b_qkv"], dtype=np.float32).reshape(3, 16 * DH)[2]
    acc += np.asarray(inputs["b_out"], dtype=np.float32)[None, :]
    acc += (bv @ np.asarray(inputs["W_out"], dtype=np.float32))[None, :]
    return acc.reshape(1, N, D), res


def kernel(**inputs):
    out, _ = run(inputs, trace=False)
    return out
